# revision 8
# baseline (speedup 1.0000x reference)
"""Trainium2 Bass kernel for nn_MemNet (scatter_memory).

Key observation: the reference initializes mem = zeros((B,S,M)) and no
operation ever breaks slot symmetry (top-k over all-equal similarities
degenerates to a uniform softmax over all S slots; synthesize is
permutation-equivariant on identical slots).  Every slot of the memory
stays identical forever, so the module collapses EXACTLY to a [B, M]
recurrence:

    ctrl_t   = [h_t, rv_t]
    iface    = ctrl_t @ W_iface[:, 2M:4M+1] + b_iface[2M:4M+1]   (wv, er, ag)
    logits_t = ctrl_t @ W_logits
    rv_{t+1} = rv_t + m_t
    m_{t+1}  = 0.99*( m_t*(1 - sigmoid(er)/S) + sigmoid(ag)*wv/S )
    if t % 16 == 0:  m += (LN(m)@Sv)@So ;  m += gelu_tanh(LN(m)@SW1)@SW2
    recon_t  = rv_{t+1} @ W_hall + b_hall

(verified to ~7e-6 relative against the jax reference).  The kernel runs
the 2-layer causal transformer for h_t, the collapsed scan, and the
memory-bound [T*B, E+M] @ W_logits projection with the vocab sharded over
the 8 NeuronCores (16.4 MB of W_logits traffic per core).
"""

import numpy as np

V, E, HID, L, NH = 32000, 512, 1024, 2, 8
B, T = 4, 128
M, S = 512, 1024
INTERVAL, DECAY = 16, 0.99
N_CORES = 8
VS = V // N_CORES          # vocab slice per core = 4000
HD = E // NH               # head dim 64
TB = T * B                 # 512 rows, row index = 4*t + b

_COMPILED = {}


def _build_program():
    import concourse.bass as bass
    import concourse.bacc as bacc
    import concourse.tile as tile
    from concourse import mybir
    from concourse.masks import make_identity

    f32 = mybir.dt.float32
    AX = mybir.AxisListType
    OP = mybir.AluOpType
    AF = mybir.ActivationFunctionType
    PSUM = bass.MemorySpace.PSUM

    nc = bacc.Bacc("TRN2", target_bir_lowering=False, debug=False,
                   num_devices=N_CORES)

    # register the LN epsilon as a const AP (activation bias needs one)
    _eps_t = nc.alloc_sbuf_tensor("const-eps-ln", [128, 1], f32)
    nc.gpsimd.memset(_eps_t.ap(), 1e-5)
    nc.const_aps.aps[(f32, 1e-5)] = _eps_t.ap()

    # ---------------- DRAM parameters ----------------
    d_x0 = nc.dram_tensor("x0", [B, T, E], f32, kind="ExternalInput")
    d_wqkv = nc.dram_tensor("Wqkv", [L, E, 3 * E], f32, kind="ExternalInput")
    d_wo = nc.dram_tensor("Wo", [L, E, E], f32, kind="ExternalInput")
    d_w1 = nc.dram_tensor("W1", [L, E, HID], f32, kind="ExternalInput")
    d_w2 = nc.dram_tensor("W2", [L, HID, E], f32, kind="ExternalInput")
    d_mask = nc.dram_tensor("maskb", [T, T], f32, kind="ExternalInput")
    d_wifh = nc.dram_tensor("W_if_h", [E, 2 * M + 1], f32, kind="ExternalInput")
    d_wifr = nc.dram_tensor("W_if_r", [M, 2 * M + 1], f32, kind="ExternalInput")
    d_bif = nc.dram_tensor("b_if", [1, 2 * M + 1], f32, kind="ExternalInput")
    d_sv = nc.dram_tensor("Sv", [M, M], f32, kind="ExternalInput")
    d_so = nc.dram_tensor("So", [M, M], f32, kind="ExternalInput")
    d_sw1 = nc.dram_tensor("SW1", [M, 128], f32, kind="ExternalInput")
    d_sw2 = nc.dram_tensor("SW2", [128, M], f32, kind="ExternalInput")
    d_whall = nc.dram_tensor("W_hall", [M, E], f32, kind="ExternalInput")
    d_bhall = nc.dram_tensor("b_hall", [1, E], f32, kind="ExternalInput")
    d_wlog = nc.dram_tensor("W_log", [E + M, VS], f32, kind="ExternalInput")

    d_logits = nc.dram_tensor("logits_s", [TB, VS], f32, kind="ExternalOutput")
    d_recon = nc.dram_tensor("recon_s", [TB, E], f32, kind="ExternalOutput")

    # scratch DRAM: precomputed per-step iface h-part in scan layout
    d_preif = nc.dram_tensor("preif", [B, T, 2 * M], f32)
    d_preag = nc.dram_tensor("preag", [B, T], f32)

    with tile.TileContext(nc) as tc:
        with (
            tc.tile_pool(name="const", bufs=1) as cpool,
            tc.tile_pool(name="persist", bufs=1) as pp,
        ):
            ident = cpool.tile([128, 128], f32, tag="ident")
            make_identity(nc, ident[:])
            ones_col = cpool.tile([1, 128], f32, tag="ones")
            nc.gpsimd.memset(ones_col[:], 1.0)

            # persistent SBUF state (~62 KB/partition)
            hsT = pp.tile([128, 4 * TB], f32, tag="hsT")     # chunk k: cols k*TB + (4t+b)
            rvh = pp.tile([128, 4 * 520], f32, tag="rvh")    # chunk k: cols k*520 + (4t+b)
            m_t = pp.tile([B, M], f32, tag="m")
            rv = pp.tile([B, M], f32, tag="rv")
            pre_ag = pp.tile([B, T], f32, tag="preag")
            w_ifr = pp.tile([128, 4 * (2 * M + 1)], f32, tag="wifr")
            sv_sb = pp.tile([128, 4 * M], f32, tag="sv")
            so_sb = pp.tile([128, 4 * M], f32, tag="so")
            sw1_sb = pp.tile([128, 4 * 128], f32, tag="sw1")
            sw2_sb = pp.tile([128, M], f32, tag="sw2")
            whall_sb = pp.tile([128, 4 * E], f32, tag="whall")
            bhall_sb = pp.tile([1, E], f32, tag="bhall")
            bif_sb = pp.tile([1, 2 * M + 1], f32, tag="bif")
            mask_sb = pp.tile([T, T], f32, tag="mask")

            nc.sync.dma_start(mask_sb[:], d_mask[:])
            nc.sync.dma_start(bif_sb[:], d_bif[:])
            nc.sync.dma_start(bhall_sb[:], d_bhall[:])
            nc.sync.dma_start(sw2_sb[:], d_sw2[:])
            for k in range(4):
                nc.sync.dma_start(w_ifr[:, k * 1025:(k + 1) * 1025],
                                  d_wifr[k * 128:(k + 1) * 128, :])
                nc.sync.dma_start(sv_sb[:, k * M:(k + 1) * M],
                                  d_sv[k * 128:(k + 1) * 128, :])
                nc.sync.dma_start(so_sb[:, k * M:(k + 1) * M],
                                  d_so[k * 128:(k + 1) * 128, :])
                nc.sync.dma_start(sw1_sb[:, k * 128:(k + 1) * 128],
                                  d_sw1[k * 128:(k + 1) * 128, :])
                nc.sync.dma_start(whall_sb[:, k * E:(k + 1) * E],
                                  d_whall[k * 128:(k + 1) * 128, :])

            nc.gpsimd.memset(m_t[:], 0.0)
            nc.gpsimd.memset(rv[:], 0.0)

            def hsT_chunk(k, b):
                return hsT[:, k * TB:(k + 1) * TB].rearrange(
                    "p (t b) -> p t b", b=4)[:, :, b]

            # ============ Phase T: transformer ============
            with (
                tc.tile_pool(name="twts", bufs=1) as twp,
                tc.tile_pool(name="tact", bufs=1) as ta,
                tc.tile_pool(name="tpsum", bufs=1, space=PSUM) as tps,
            ):
                xs = [ta.tile([T, E], f32, tag=f"x{b}", name=f"x{b}") for b in range(B)]
                for b in range(B):
                    nc.sync.dma_start(xs[b][:], d_x0[b])

                def layer_norm(x_ap, P, F):
                    s1 = ta.tile([P, 1], f32, tag="ln_s1")
                    nc.vector.tensor_reduce(s1[:], x_ap, AX.X, op=OP.add)
                    mu = ta.tile([P, 1], f32, tag="ln_mu")
                    nc.vector.tensor_scalar(mu[:], s1[:], 1.0 / F, None, OP.mult)
                    xc = ta.tile([P, F], f32, tag="ln_xc")
                    nc.vector.tensor_scalar(xc[:], x_ap, mu[:], None, OP.subtract)
                    sq = ta.tile([P, F], f32, tag="ln_sq")
                    s2 = ta.tile([P, 1], f32, tag="ln_s2")
                    nc.vector.scalar_tensor_tensor(sq[:], xc[:], 1.0, xc[:],
                                                   OP.mult, OP.mult,
                                                   accum_out=s2[:])
                    var = ta.tile([P, 1], f32, tag="ln_var")
                    nc.vector.tensor_scalar(var[:], s2[:], 1.0 / F, None, OP.mult)
                    sd = ta.tile([P, 1], f32, tag="ln_sd")
                    nc.scalar.activation(sd[:], var[:], AF.Sqrt, bias=1e-5)
                    ri = ta.tile([P, 1], f32, tag="ln_ri")
                    nc.vector.reciprocal(ri[:], sd[:])
                    xh = ta.tile([P, F], f32, tag="ln_xh")
                    nc.vector.tensor_scalar(xh[:], xc[:], ri[:], None, OP.mult)
                    return xh

                def transpose_to(src_ap, dst_ap, P, Fc):
                    """PE-transpose src [P, Fc*128] into dst [128, Fc*P]."""
                    for k in range(Fc):
                        pt = tps.tile([128, 128], f32, tag="ptr")
                        nc.tensor.transpose(pt[:, 0:P],
                                            src_ap[:, k * 128:(k + 1) * 128],
                                            ident[0:P, 0:P])
                        nc.vector.tensor_copy(dst_ap[:, k * P:(k + 1) * P],
                                              pt[:, 0:P])

                for l in range(L):
                    wqkv = twp.tile([128, 4 * 3 * E], f32, tag="wqkv")
                    wo_w = twp.tile([128, 4 * E], f32, tag="wo")
                    w1_w = twp.tile([128, 4 * HID], f32, tag="w1")
                    w2_w = twp.tile([128, 8 * E], f32, tag="w2")
                    for k in range(4):
                        nc.sync.dma_start(wqkv[:, k * 3 * E:(k + 1) * 3 * E],
                                          d_wqkv[l, k * 128:(k + 1) * 128, :])
                        nc.sync.dma_start(wo_w[:, k * E:(k + 1) * E],
                                          d_wo[l, k * 128:(k + 1) * 128, :])
                        nc.sync.dma_start(w1_w[:, k * HID:(k + 1) * HID],
                                          d_w1[l, k * 128:(k + 1) * 128, :])
                    for k in range(8):
                        nc.sync.dma_start(w2_w[:, k * E:(k + 1) * E],
                                          d_w2[l, k * 128:(k + 1) * 128, :])

                    for b in range(B):
                        x = xs[b]
                        xh = layer_norm(x[:], T, E)
                        xhT = ta.tile([128, 4 * T], f32, tag="xhT")
                        transpose_to(xh[:], xhT[:], T, 4)
                        qkv = ta.tile([T, 3 * E], f32, tag="qkv")
                        for n in range(3):
                            pq = tps.tile([128, 512], f32, tag="mm512")
                            for k in range(4):
                                nc.tensor.matmul(
                                    pq[:], xhT[:, k * T:(k + 1) * T],
                                    wqkv[:, k * 3 * E + n * 512:
                                         k * 3 * E + (n + 1) * 512],
                                    start=(k == 0), stop=(k == 3))
                            nc.vector.tensor_copy(qkv[:, n * 512:(n + 1) * 512],
                                                  pq[:])
                        attn = ta.tile([T, E], f32, tag="attn")
                        for h in range(NH):
                            qT = ta.tile([HD, T], f32, tag="qT")
                            pt1 = tps.tile([128, 128], f32, tag="ptr")
                            nc.tensor.transpose(pt1[0:HD, 0:T],
                                                qkv[:, h * HD:(h + 1) * HD],
                                                ident[0:T, 0:T])
                            nc.vector.tensor_copy(qT[:], pt1[0:HD, 0:T])
                            kT = ta.tile([HD, T], f32, tag="kT")
                            pt2 = tps.tile([128, 128], f32, tag="ptr")
                            nc.tensor.transpose(pt2[0:HD, 0:T],
                                                qkv[:, E + h * HD:E + (h + 1) * HD],
                                                ident[0:T, 0:T])
                            nc.vector.tensor_copy(kT[:], pt2[0:HD, 0:T])
                            psc = tps.tile([T, T], f32, tag="psc")
                            nc.tensor.matmul(psc[:], qT[:], kT[:], start=True, stop=True)
                            scm = ta.tile([T, T], f32, tag="scm")
                            nc.vector.scalar_tensor_tensor(
                                scm[:], psc[:], float(1.0 / np.sqrt(HD)),
                                mask_sb[:], OP.mult, OP.add)
                            p_e = ta.tile([T, T], f32, tag="p_e")
                            zrow = ta.tile([T, 1], f32, tag="zrow")
                            nc.scalar.activation(p_e[:], scm[:], AF.Exp,
                                                 accum_out=zrow[:])
                            rz = ta.tile([T, 1], f32, tag="rz")
                            nc.vector.reciprocal(rz[:], zrow[:])
                            pT = ta.tile([T, T], f32, tag="pT")
                            pt3 = tps.tile([128, 128], f32, tag="ptr")
                            nc.tensor.transpose(pt3[:, 0:T], p_e[:],
                                                ident[0:T, 0:T])
                            nc.vector.tensor_copy(pT[:], pt3[:, 0:T])
                            pav = tps.tile([T, HD], f32, tag="pav")
                            nc.tensor.matmul(
                                pav[:], pT[:],
                                qkv[:, 2 * E + h * HD:2 * E + (h + 1) * HD],
                                start=True, stop=True)
                            nc.vector.tensor_scalar(attn[:, h * HD:(h + 1) * HD],
                                                    pav[:], rz[:], None, OP.mult)
                        attnT = ta.tile([128, 4 * T], f32, tag="xhT")
                        transpose_to(attn[:], attnT[:], T, 4)
                        po = tps.tile([128, 512], f32, tag="mm512")
                        for k in range(4):
                            nc.tensor.matmul(po[:], attnT[:, k * T:(k + 1) * T],
                                             wo_w[:, k * E:(k + 1) * E],
                                             start=(k == 0), stop=(k == 3))
                        nc.vector.tensor_tensor(x[:], x[:], po[:], OP.add)
                        yh = layer_norm(x[:], T, E)
                        yhT = ta.tile([128, 4 * T], f32, tag="xhT")
                        transpose_to(yh[:], yhT[:], T, 4)
                        h1 = ta.tile([T, HID], f32, tag="h1")
                        for n in range(2):
                            ph = tps.tile([128, 512], f32, tag="mm512")
                            for k in range(4):
                                nc.tensor.matmul(
                                    ph[:], yhT[:, k * T:(k + 1) * T],
                                    w1_w[:, k * HID + n * 512:
                                         k * HID + (n + 1) * 512],
                                    start=(k == 0), stop=(k == 3))
                            h2_ = ta.tile([T, 512], f32, tag="g_h2")
                            nc.scalar.activation(h2_[:], ph[:], AF.Square)
                            c_ = ta.tile([T, 512], f32, tag="g_c")
                            nc.vector.tensor_scalar(c_[:], h2_[:], 0.044715, 1.0,
                                                    OP.mult, OP.add)
                            dd = ta.tile([T, 512], f32, tag="g_d")
                            nc.vector.tensor_tensor(dd[:], c_[:], ph[:], OP.mult)
                            th = ta.tile([T, 512], f32, tag="g_t")
                            nc.scalar.activation(th[:], dd[:], AF.Tanh,
                                                 scale=0.7978845608028654)
                            ff = ta.tile([T, 512], f32, tag="g_f")
                            nc.vector.tensor_scalar(ff[:], th[:], 1.0, 0.5,
                                                    OP.add, OP.mult)
                            nc.vector.tensor_tensor(h1[:, n * 512:(n + 1) * 512],
                                                    ff[:], ph[:], OP.mult)
                        h1T = ta.tile([128, 8 * T], f32, tag="h1T")
                        transpose_to(h1[:], h1T[:], T, 8)
                        ph2 = tps.tile([128, 512], f32, tag="mm512")
                        for k in range(8):
                            nc.tensor.matmul(ph2[:], h1T[:, k * T:(k + 1) * T],
                                             w2_w[:, k * E:(k + 1) * E],
                                             start=(k == 0), stop=(k == 7))
                        nc.vector.tensor_tensor(x[:], x[:], ph2[:], OP.add)

                # hsT (interleaved 4t+b) + pre_iface h-part
                wifh_sb = twp.tile([128, 4 * 1025], f32, tag="wqkv")  # reuse slot
                for k in range(4):
                    nc.sync.dma_start(wifh_sb[:, k * 1025:(k + 1) * 1025],
                                      d_wifh[k * 128:(k + 1) * 128, :])
                for b in range(B):
                    for k in range(4):
                        pt = tps.tile([128, 128], f32, tag="ptr")
                        nc.tensor.transpose(pt[:, 0:T],
                                            xs[b][:, k * 128:(k + 1) * 128],
                                            ident[0:T, 0:T])
                        nc.vector.tensor_copy(hsT_chunk(k, b), pt[:, 0:T])
                    tmp = ta.tile([T, 2 * M], f32, tag="preif_tmp")
                    for n in range(2):
                        ppi = tps.tile([128, 512], f32, tag="mm512")
                        for k in range(4):
                            nc.tensor.matmul(
                                ppi[:], hsT_chunk(k, b),
                                wifh_sb[:, k * 1025 + n * 512:
                                        k * 1025 + (n + 1) * 512],
                                start=(k == 0), stop=False)
                        nc.tensor.matmul(ppi[:], ones_col[:],
                                         bif_sb[:, n * 512:(n + 1) * 512],
                                         start=False, stop=True)
                        nc.vector.tensor_copy(tmp[:, n * 512:(n + 1) * 512],
                                              ppi[:])
                    nc.sync.dma_start(d_preif[b], tmp[:])
                    pag = tps.tile([T, 1], f32, tag="pag")
                    for k in range(4):
                        nc.tensor.matmul(
                            pag[:], hsT_chunk(k, b),
                            wifh_sb[:, k * 1025 + 1024:k * 1025 + 1025],
                            start=(k == 0), stop=False)
                    nc.tensor.matmul(pag[:], ones_col[:], bif_sb[:, 1024:1025],
                                     start=False, stop=True)
                    tmpa = ta.tile([T, 1], f32, tag="preag_tmp")
                    nc.vector.tensor_copy(tmpa[:], pag[:])
                    nc.sync.dma_start(d_preag[b], tmpa[:, 0:1])

            nc.sync.dma_start(pre_ag[:], d_preag[:])

            # ============ Phase S: the collapsed scan ============
            with (
                tc.tile_pool(name="scana", bufs=2) as sa,
                tc.tile_pool(name="spsA", bufs=1, space=PSUM) as spsA,
                tc.tile_pool(name="spsB", bufs=2, space=PSUM) as spsB,
            ):
                def rv_to_hist(t):
                    prt = spsB.tile([128, 16], f32, tag="prvt")
                    for k in range(4):
                        nc.tensor.transpose(prt[:, 4 * k:4 * k + 4],
                                            rv[:, k * 128:(k + 1) * 128],
                                            ident[0:4, 0:4])
                    for k in range(4):
                        nc.vector.tensor_copy(
                            rvh[:, k * 520 + 4 * t:k * 520 + 4 * t + 4],
                            prt[:, 4 * k:4 * k + 4])

                def synth_ln(src):
                    s1 = sa.tile([B, 1], f32, tag="sln_s1")
                    nc.vector.tensor_reduce(s1[:], src[:], AX.X, op=OP.add)
                    mu = sa.tile([B, 1], f32, tag="sln_mu")
                    nc.vector.tensor_scalar(mu[:], s1[:], 1.0 / M, None, OP.mult)
                    xc = sa.tile([B, M], f32, tag="sln_xc")
                    nc.vector.tensor_scalar(xc[:], src[:], mu[:], None,
                                            OP.subtract)
                    s2 = sa.tile([B, 1], f32, tag="sln_s2")
                    sq = sa.tile([B, M], f32, tag="sln_sq")
                    nc.vector.scalar_tensor_tensor(sq[:], xc[:], 1.0, xc[:],
                                                   OP.mult, OP.mult,
                                                   accum_out=s2[:])
                    var = sa.tile([B, 1], f32, tag="sln_var")
                    nc.vector.tensor_scalar(var[:], s2[:], 1.0 / M, None, OP.mult)
                    sd = sa.tile([B, 1], f32, tag="sln_sd")
                    nc.scalar.activation(sd[:], var[:], AF.Sqrt, bias=1e-5)
                    ri = sa.tile([B, 1], f32, tag="sln_ri")
                    nc.vector.reciprocal(ri[:], sd[:])
                    xh = sa.tile([B, M], f32, tag="sln_xh")
                    nc.vector.tensor_scalar(xh[:], xc[:], ri[:], None, OP.mult)
                    return xh

                def small_T(src_ap, Fc, tag):
                    prt = spsB.tile([128, 16], f32, tag="prvt")
                    out = sa.tile([128, 4 * Fc], f32, tag=tag)
                    for k in range(Fc):
                        nc.tensor.transpose(prt[:, 4 * k:4 * k + 4],
                                            src_ap[:, k * 128:(k + 1) * 128],
                                            ident[0:4, 0:4])
                    nc.vector.tensor_copy(out[:], prt[:, 0:4 * Fc])
                    return out

                for t in range(T):
                    rv_to_hist(t)
                    pifq = sa.tile([B, 2 * M], f32, tag="pifq")
                    nc.sync.dma_start(pifq[:], d_preif[:, t, :])
                    psif = spsA.tile([B, 2 * M], f32, tag="psif")
                    for n in range(2):
                        for k in range(4):
                            nc.tensor.matmul(
                                psif[:, n * 512:(n + 1) * 512],
                                rvh[:, k * 520 + 4 * t:k * 520 + 4 * t + 4],
                                w_ifr[:, k * 1025 + n * 512:
                                      k * 1025 + (n + 1) * 512],
                                start=(k == 0), stop=(k == 3))
                    psag = spsA.tile([B, 1], f32, tag="psag")
                    for k in range(4):
                        nc.tensor.matmul(
                            psag[:],
                            rvh[:, k * 520 + 4 * t:k * 520 + 4 * t + 4],
                            w_ifr[:, k * 1025 + 1024:k * 1025 + 1025],
                            start=(k == 0), stop=(k == 3))
                    ifc = sa.tile([B, 2 * M], f32, tag="ifc")
                    nc.vector.tensor_tensor(ifc[:], psif[:], pifq[:], OP.add)
                    aga = sa.tile([B, 1], f32, tag="aga")
                    nc.vector.tensor_tensor(aga[:], psag[:],
                                            pre_ag[:, t:t + 1], OP.add)
                    ers = sa.tile([B, M], f32, tag="ers")
                    nc.scalar.activation(ers[:], ifc[:, M:2 * M], AF.Sigmoid)
                    gts = sa.tile([B, 1], f32, tag="gts")
                    nc.scalar.activation(gts[:], aga[:], AF.Sigmoid)
                    nc.vector.tensor_tensor(rv[:], rv[:], m_t[:], OP.add)
                    u1 = sa.tile([B, M], f32, tag="u1")
                    nc.vector.scalar_tensor_tensor(u1[:], m_t[:], -DECAY / S,
                                                   ers[:], OP.mult, OP.mult)
                    u2 = sa.tile([B, M], f32, tag="u2")
                    nc.vector.scalar_tensor_tensor(u2[:], m_t[:], DECAY, u1[:],
                                                   OP.mult, OP.add)
                    u3 = sa.tile([B, M], f32, tag="u3")
                    nc.vector.tensor_scalar(u3[:], ifc[:, 0:M], gts[:],
                                            DECAY / S, OP.mult, OP.mult)
                    nc.vector.tensor_tensor(m_t[:], u2[:], u3[:], OP.add)

                    if t % INTERVAL == 0:
                        xh = synth_ln(m_t)
                        xhT = small_T(xh[:], 4, "xhT_s")
                        py = spsA.tile([B, M], f32, tag="psy")
                        for k in range(4):
                            nc.tensor.matmul(py[:], xhT[:, 4 * k:4 * k + 4],
                                             sv_sb[:, k * M:(k + 1) * M],
                                             start=(k == 0), stop=(k == 3))
                        y1 = sa.tile([B, M], f32, tag="y1")
                        nc.vector.tensor_copy(y1[:], py[:])
                        y1T = small_T(y1[:], 4, "y1T_s")
                        py2 = spsA.tile([B, M], f32, tag="psy")
                        for k in range(4):
                            nc.tensor.matmul(py2[:], y1T[:, 4 * k:4 * k + 4],
                                             so_sb[:, k * M:(k + 1) * M],
                                             start=(k == 0), stop=(k == 3))
                        nc.vector.tensor_tensor(m_t[:], m_t[:], py2[:], OP.add)
                        x2 = synth_ln(m_t)
                        x2T = small_T(x2[:], 4, "x2T_s")
                        ph1 = spsA.tile([B, 128], f32, tag="psh")
                        for k in range(4):
                            nc.tensor.matmul(ph1[:], x2T[:, 4 * k:4 * k + 4],
                                             sw1_sb[:, k * 128:(k + 1) * 128],
                                             start=(k == 0), stop=(k == 3))
                        g2 = sa.tile([B, 128], f32, tag="g2")
                        nc.scalar.activation(g2[:], ph1[:], AF.Square)
                        gc = sa.tile([B, 128], f32, tag="gc")
                        nc.vector.tensor_scalar(gc[:], g2[:], 0.044715, 1.0,
                                                OP.mult, OP.add)
                        gd = sa.tile([B, 128], f32, tag="gd")
                        nc.vector.tensor_tensor(gd[:], gc[:], ph1[:], OP.mult)
                        gt_ = sa.tile([B, 128], f32, tag="gt_")
                        nc.scalar.activation(gt_[:], gd[:], AF.Tanh,
                                             scale=0.7978845608028654)
                        gf = sa.tile([B, 128], f32, tag="gf")
                        nc.vector.tensor_scalar(gf[:], gt_[:], 1.0, 0.5,
                                                OP.add, OP.mult)
                        gg = sa.tile([B, 128], f32, tag="gg")
                        nc.vector.tensor_tensor(gg[:], gf[:], ph1[:], OP.mult)
                        ggT = small_T(gg[:], 1, "ggT_s")
                        pm2 = spsA.tile([B, M], f32, tag="psy")
                        nc.tensor.matmul(pm2[:], ggT[:, 0:4], sw2_sb[:],
                                         start=True, stop=True)
                        nc.vector.tensor_tensor(m_t[:], m_t[:], pm2[:], OP.add)

                rv_to_hist(T)

            # ============ Phase L: logits + recon ============
            with (
                tc.tile_pool(name="lg", bufs=3) as lg,
                tc.tile_pool(name="lgo", bufs=2) as lgo,
                tc.tile_pool(name="lpsum", bufs=1, space=PSUM) as lps,
            ):
                def ctrl_chunk(k, mt):
                    if k < 4:
                        return hsT[:, k * TB + 128 * mt:k * TB + 128 * (mt + 1)]
                    return rvh[:, (k - 4) * 520 + 128 * mt:
                               (k - 4) * 520 + 128 * (mt + 1)]

                nsizes = [512] * 7 + [VS - 7 * 512]
                ncol = 0
                for n, nsz in enumerate(nsizes):
                    plgs = [lps.tile([128, 512], f32, tag=f"plg{mt}", name=f"plg{mt}")
                            for mt in range(4)]
                    for k in range(8):
                        wl = lg.tile([128, 512], f32, tag="wl")
                        nc.sync.dma_start(wl[:, 0:nsz],
                                          d_wlog[k * 128:(k + 1) * 128,
                                                 ncol:ncol + nsz])
                        for mt in range(4):
                            nc.tensor.matmul(plgs[mt][:, 0:nsz],
                                             ctrl_chunk(k, mt), wl[:, 0:nsz],
                                             start=(k == 0), stop=(k == 7))
                    for mt in range(4):
                        ob = lgo.tile([128, 512], f32, tag="ob")
                        nc.vector.tensor_copy(ob[:, 0:nsz], plgs[mt][:, 0:nsz])
                        nc.sync.dma_start(
                            d_logits[128 * mt:128 * (mt + 1), ncol:ncol + nsz],
                            ob[:, 0:nsz])
                    ncol += nsz

                for mt in range(4):
                    prc = lps.tile([128, E], f32, tag="prc")
                    for k in range(4):
                        nc.tensor.matmul(
                            prc[:],
                            rvh[:, k * 520 + 4 + 128 * mt:
                                k * 520 + 4 + 128 * (mt + 1)],
                            whall_sb[:, k * E:(k + 1) * E],
                            start=(k == 0), stop=False)
                    nc.tensor.matmul(prc[:], ones_col[:], bhall_sb[:],
                                     start=False, stop=True)
                    orc = lgo.tile([128, E], f32, tag="orc")
                    nc.vector.tensor_copy(orc[:], prc[:])
                    nc.sync.dma_start(d_recon[128 * mt:128 * (mt + 1), :],
                                      orc[:])

    nc.compile()
    return nc


def kernel(**inputs):
    inputs = {k: np.asarray(v) for k, v in inputs.items()}
    x0 = (inputs["embed"][inputs["input_seq"].astype(np.int64)]
          + inputs["pos"][None, :T, :]).astype(np.float32)
    maskb = np.where(np.tril(np.ones((T, T), bool)), 0.0,
                     -10000.0).astype(np.float32)
    W_iface = inputs["W_iface"].astype(np.float32)
    base = {
        "x0": x0,
        "Wqkv": inputs["Wqkv"].astype(np.float32),
        "Wo": inputs["Wo"].astype(np.float32),
        "W1": inputs["W1"].astype(np.float32),
        "W2": inputs["W2"].astype(np.float32),
        "maskb": maskb,
        "W_if_h": np.ascontiguousarray(W_iface[:E, 2 * M:]),
        "W_if_r": np.ascontiguousarray(W_iface[E:, 2 * M:]),
        "b_if": inputs["b_iface"][2 * M:].reshape(1, -1).astype(np.float32),
        "Sv": inputs["Sv"][0].astype(np.float32),
        "So": inputs["So"][0].astype(np.float32),
        "SW1": inputs["SW1"][0].astype(np.float32),
        "SW2": inputs["SW2"][0].astype(np.float32),
        "W_hall": inputs["W_hall"].astype(np.float32),
        "b_hall": inputs["b_hall"].reshape(1, -1).astype(np.float32),
    }
    W_logits = inputs["W_logits"].astype(np.float32)
    in_maps = []
    for c in range(N_CORES):
        im = dict(base)
        im["W_log"] = np.ascontiguousarray(W_logits[:, c * VS:(c + 1) * VS])
        in_maps.append(im)

    if "nc" not in _COMPILED:
        _COMPILED["nc"] = _build_program()
    from concourse.bass_utils import run_bass_kernel_spmd
    res = run_bass_kernel_spmd(_COMPILED["nc"], in_maps,
                               core_ids=list(range(N_CORES)))
    globals()["LAST_RES"] = res
    logits = np.empty((B, T, V), np.float32)
    for c in range(N_CORES):
        sl = res.results[c]["logits_s"]                  # [TB, VS], row 4t+b
        logits[:, :, c * VS:(c + 1) * VS] = \
            sl.reshape(T, B, VS).transpose(1, 0, 2)
    recon = res.results[0]["recon_s"].reshape(T, B, E).transpose(1, 0, 2)
    return logits, np.ascontiguousarray(recon)


# revision 11
# speedup vs baseline: 1.4844x; 1.4844x over previous
"""Trainium2 Bass kernel for nn_MemNet (scatter_memory).

Key observation: the reference initializes mem = zeros((B,S,M)) and no
operation ever breaks slot symmetry (top-k over all-equal similarities
degenerates to a uniform softmax over all S slots; synthesize is
permutation-equivariant on identical slots).  Every slot of the memory
stays identical forever, so the module collapses EXACTLY to a [B, M]
recurrence:

    ctrl_t   = [h_t, rv_t]
    iface    = ctrl_t @ W_iface[:, 2M:4M+1] + b_iface[2M:4M+1]   (wv, er, ag)
    logits_t = ctrl_t @ W_logits
    rv_{t+1} = rv_t + m_t
    m_{t+1}  = 0.99*( m_t*(1 - sigmoid(er)/S) + sigmoid(ag)*wv/S )
    if t % 16 == 0:  m += (LN(m)@Sv)@So ;  m += gelu_tanh(LN(m)@SW1)@SW2
    recon_t  = rv_{t+1} @ W_hall + b_hall

(verified to ~7e-6 relative against the jax reference).  The kernel runs
the 2-layer causal transformer for h_t, the collapsed scan, and the
memory-bound [T*B, E+M] @ W_logits projection with the vocab sharded over
the 8 NeuronCores (16.4 MB of W_logits traffic per core).
"""

import numpy as np
import ml_dtypes

BF16 = ml_dtypes.bfloat16

V, E, HID, L, NH = 32000, 512, 1024, 2, 8
B, T = 4, 128
M, S = 512, 1024
INTERVAL, DECAY = 16, 0.99
N_CORES = 8
VS = V // N_CORES          # vocab slice per core = 4000
HD = E // NH               # head dim 64
TB = T * B                 # 512 rows, row index = 4*t + b

_COMPILED = {}


def _build_program():
    import concourse.bass as bass
    import concourse.bacc as bacc
    import concourse.tile as tile
    from concourse import mybir
    from concourse.masks import make_identity

    f32 = mybir.dt.float32
    bf16 = mybir.dt.bfloat16
    AX = mybir.AxisListType
    OP = mybir.AluOpType
    AF = mybir.ActivationFunctionType
    PSUM = bass.MemorySpace.PSUM

    nc = bacc.Bacc("TRN2", target_bir_lowering=False, debug=False,
                   num_devices=N_CORES)

    # register the LN epsilon as a const AP (activation bias needs one)
    _eps_t = nc.alloc_sbuf_tensor("const-eps-ln", [128, 1], f32)
    nc.gpsimd.memset(_eps_t.ap(), 1e-5)
    nc.const_aps.aps[(f32, 1e-5)] = _eps_t.ap()

    # ---------------- DRAM parameters ----------------
    d_x0 = nc.dram_tensor("x0", [B, T, E], f32, kind="ExternalInput")
    d_wqkv = nc.dram_tensor("Wqkv", [L, E, 3 * E], bf16, kind="ExternalInput")
    d_wo = nc.dram_tensor("Wo", [L, E, E], bf16, kind="ExternalInput")
    d_w1 = nc.dram_tensor("W1", [L, E, HID], bf16, kind="ExternalInput")
    d_w2 = nc.dram_tensor("W2", [L, HID, E], bf16, kind="ExternalInput")
    d_mask = nc.dram_tensor("maskb", [T, T], f32, kind="ExternalInput")
    d_wifh = nc.dram_tensor("W_if_h", [E, 2 * M + 1], bf16, kind="ExternalInput")
    d_wifr = nc.dram_tensor("W_if_r", [M, 2 * M + 1], bf16, kind="ExternalInput")
    d_bif = nc.dram_tensor("b_if", [1, 2 * M + 1], bf16, kind="ExternalInput")
    d_sv = nc.dram_tensor("Sv", [M, M], bf16, kind="ExternalInput")
    d_so = nc.dram_tensor("So", [M, M], bf16, kind="ExternalInput")
    d_sw1 = nc.dram_tensor("SW1", [M, 128], bf16, kind="ExternalInput")
    d_sw2 = nc.dram_tensor("SW2", [128, M], bf16, kind="ExternalInput")
    d_whall = nc.dram_tensor("W_hall", [M, E], bf16, kind="ExternalInput")
    d_bhall = nc.dram_tensor("b_hall", [1, E], bf16, kind="ExternalInput")
    d_wlog = nc.dram_tensor("W_log", [E + M, VS], bf16, kind="ExternalInput")

    d_logits = nc.dram_tensor("logits_s", [TB, VS], f32, kind="ExternalOutput")
    d_recon = nc.dram_tensor("recon_s", [TB, E], f32, kind="ExternalOutput")

    # scratch DRAM: precomputed per-step iface h-part in scan layout
    d_preif = nc.dram_tensor("preif", [B, T, 2 * M], bf16)
    d_preag = nc.dram_tensor("preag", [B, T], f32)

    with tile.TileContext(nc) as tc:
        with (
            tc.tile_pool(name="const", bufs=1) as cpool,
            tc.tile_pool(name="persist", bufs=1) as pp,
        ):
            ident = cpool.tile([128, 128], f32, tag="ident")
            make_identity(nc, ident[:])
            ones_col = cpool.tile([1, 128], bf16, tag="ones")
            nc.gpsimd.memset(ones_col[:], 1.0)
            ident_bf = cpool.tile([128, 128], bf16, tag="identbf")
            nc.vector.tensor_copy(ident_bf[:], ident[:])

            # persistent SBUF state (~62 KB/partition)
            hsT = pp.tile([128, 4 * TB], bf16, tag="hsT")     # chunk k: cols k*TB + (4t+b)
            rvh = pp.tile([128, 4 * 520], bf16, tag="rvh")    # chunk k: cols k*520 + (4t+b)
            m_t = pp.tile([B, M], f32, tag="m")
            rv = pp.tile([B, M], f32, tag="rv")
            pre_ag = pp.tile([B, T], f32, tag="preag")
            w_ifr = pp.tile([128, 4 * (2 * M + 1)], bf16, tag="wifr")
            sv_sb = pp.tile([128, 4 * M], bf16, tag="sv")
            so_sb = pp.tile([128, 4 * M], bf16, tag="so")
            sw1_sb = pp.tile([128, 4 * 128], bf16, tag="sw1")
            sw2_sb = pp.tile([128, M], bf16, tag="sw2")
            whall_sb = pp.tile([128, 4 * E], bf16, tag="whall")
            bhall_sb = pp.tile([1, E], bf16, tag="bhall")
            bif_sb = pp.tile([1, 2 * M + 1], bf16, tag="bif")
            mask_sb = pp.tile([T, T], f32, tag="mask")

            nc.sync.dma_start(mask_sb[:], d_mask[:])
            nc.sync.dma_start(bif_sb[:], d_bif[:])
            nc.sync.dma_start(bhall_sb[:], d_bhall[:])
            nc.sync.dma_start(sw2_sb[:], d_sw2[:])
            for k in range(4):
                nc.sync.dma_start(w_ifr[:, k * 1025:(k + 1) * 1025],
                                  d_wifr[k * 128:(k + 1) * 128, :])
                nc.sync.dma_start(sv_sb[:, k * M:(k + 1) * M],
                                  d_sv[k * 128:(k + 1) * 128, :])
                nc.sync.dma_start(so_sb[:, k * M:(k + 1) * M],
                                  d_so[k * 128:(k + 1) * 128, :])
                nc.sync.dma_start(sw1_sb[:, k * 128:(k + 1) * 128],
                                  d_sw1[k * 128:(k + 1) * 128, :])
                nc.sync.dma_start(whall_sb[:, k * E:(k + 1) * E],
                                  d_whall[k * 128:(k + 1) * 128, :])

            nc.gpsimd.memset(m_t[:], 0.0)
            nc.gpsimd.memset(rv[:], 0.0)

            def hsT_chunk(k, b):
                return hsT[:, k * TB:(k + 1) * TB].rearrange(
                    "p (t b) -> p t b", b=4)[:, :, b]

            # ============ Phase T: transformer ============
            with (
                tc.tile_pool(name="twts", bufs=1) as twp,
                tc.tile_pool(name="tact", bufs=1) as ta,
                tc.tile_pool(name="tpsum", bufs=1, space=PSUM) as tps,
            ):
                xs = [ta.tile([T, E], f32, tag=f"x{b}", name=f"x{b}") for b in range(B)]
                for b in range(B):
                    nc.sync.dma_start(xs[b][:], d_x0[b])

                def layer_norm(x_ap, P, F):
                    s1 = ta.tile([P, 1], f32, tag="ln_s1")
                    nc.vector.tensor_reduce(s1[:], x_ap, AX.X, op=OP.add)
                    mu = ta.tile([P, 1], f32, tag="ln_mu")
                    nc.vector.tensor_scalar(mu[:], s1[:], 1.0 / F, None, OP.mult)
                    xc = ta.tile([P, F], f32, tag="ln_xc")
                    nc.vector.tensor_scalar(xc[:], x_ap, mu[:], None, OP.subtract)
                    sq = ta.tile([P, F], f32, tag="ln_sq")
                    s2 = ta.tile([P, 1], f32, tag="ln_s2")
                    nc.vector.scalar_tensor_tensor(sq[:], xc[:], 1.0, xc[:],
                                                   OP.mult, OP.mult,
                                                   accum_out=s2[:])
                    var = ta.tile([P, 1], f32, tag="ln_var")
                    nc.vector.tensor_scalar(var[:], s2[:], 1.0 / F, None, OP.mult)
                    sd = ta.tile([P, 1], f32, tag="ln_sd")
                    nc.scalar.activation(sd[:], var[:], AF.Sqrt, bias=1e-5)
                    ri = ta.tile([P, 1], f32, tag="ln_ri")
                    nc.vector.reciprocal(ri[:], sd[:])
                    xh = ta.tile([P, F], f32, tag="ln_xh")
                    nc.vector.tensor_scalar(xh[:], xc[:], ri[:], None, OP.mult)
                    return xh

                def transpose_to(src_ap, dst_ap, P, Fc, idd=None):
                    """PE-transpose src [P, Fc*128] into dst [128, Fc*P]."""
                    if idd is None:
                        idd = ident
                    for k in range(Fc):
                        pt = tps.tile([128, 128],
                                      f32 if idd is ident else bf16, tag="ptr")
                        nc.tensor.transpose(pt[:, 0:P],
                                            src_ap[:, k * 128:(k + 1) * 128],
                                            idd[0:P, 0:P])
                        nc.vector.tensor_copy(dst_ap[:, k * P:(k + 1) * P],
                                              pt[:, 0:P])

                for l in range(L):
                    wqkv = twp.tile([128, 4 * 3 * E], bf16, tag="wqkv")
                    wo_w = twp.tile([128, 4 * E], bf16, tag="wo")
                    w1_w = twp.tile([128, 4 * HID], bf16, tag="w1")
                    w2_w = twp.tile([128, 8 * E], bf16, tag="w2")
                    for k in range(4):
                        nc.sync.dma_start(wqkv[:, k * 3 * E:(k + 1) * 3 * E],
                                          d_wqkv[l, k * 128:(k + 1) * 128, :])
                        nc.sync.dma_start(wo_w[:, k * E:(k + 1) * E],
                                          d_wo[l, k * 128:(k + 1) * 128, :])
                        nc.sync.dma_start(w1_w[:, k * HID:(k + 1) * HID],
                                          d_w1[l, k * 128:(k + 1) * 128, :])
                    for k in range(8):
                        nc.sync.dma_start(w2_w[:, k * E:(k + 1) * E],
                                          d_w2[l, k * 128:(k + 1) * 128, :])

                    for b in range(B):
                        x = xs[b]
                        xh = layer_norm(x[:], T, E)
                        xhT = ta.tile([128, 4 * T], bf16, tag="xhT")
                        transpose_to(xh[:], xhT[:], T, 4)
                        qkv = ta.tile([T, 3 * E], bf16, tag="qkv")
                        for n in range(3):
                            pq = tps.tile([128, 512], f32, tag="mm512")
                            for k in range(4):
                                nc.tensor.matmul(
                                    pq[:], xhT[:, k * T:(k + 1) * T],
                                    wqkv[:, k * 3 * E + n * 512:
                                         k * 3 * E + (n + 1) * 512],
                                    start=(k == 0), stop=(k == 3))
                            nc.vector.tensor_copy(qkv[:, n * 512:(n + 1) * 512],
                                                  pq[:])
                        attn = ta.tile([T, E], bf16, tag="attn")
                        for h in range(NH):
                            qT = ta.tile([HD, T], bf16, tag="qT")
                            pt1 = tps.tile([128, 128], bf16, tag="ptr")
                            nc.tensor.transpose(pt1[0:HD, 0:T],
                                                qkv[:, h * HD:(h + 1) * HD],
                                                ident_bf[0:T, 0:T])
                            nc.vector.tensor_copy(qT[:], pt1[0:HD, 0:T])
                            kT = ta.tile([HD, T], bf16, tag="kT")
                            pt2 = tps.tile([128, 128], bf16, tag="ptr")
                            nc.tensor.transpose(pt2[0:HD, 0:T],
                                                qkv[:, E + h * HD:E + (h + 1) * HD],
                                                ident_bf[0:T, 0:T])
                            nc.vector.tensor_copy(kT[:], pt2[0:HD, 0:T])
                            psc = tps.tile([T, T], f32, tag="psc")
                            nc.tensor.matmul(psc[:], qT[:], kT[:], start=True, stop=True)
                            scm = ta.tile([T, T], f32, tag="scm")
                            nc.vector.scalar_tensor_tensor(
                                scm[:], psc[:], float(1.0 / np.sqrt(HD)),
                                mask_sb[:], OP.mult, OP.add)
                            p_e = ta.tile([T, T], bf16, tag="p_e")
                            zrow = ta.tile([T, 1], f32, tag="zrow")
                            nc.scalar.activation(p_e[:], scm[:], AF.Exp,
                                                 accum_out=zrow[:])
                            rz = ta.tile([T, 1], f32, tag="rz")
                            nc.vector.reciprocal(rz[:], zrow[:])
                            pT = ta.tile([T, T], bf16, tag="pT")
                            pt3 = tps.tile([128, 128], bf16, tag="ptr")
                            nc.tensor.transpose(pt3[:, 0:T], p_e[:],
                                                ident_bf[0:T, 0:T])
                            nc.vector.tensor_copy(pT[:], pt3[:, 0:T])
                            pav = tps.tile([T, HD], f32, tag="pav")
                            nc.tensor.matmul(
                                pav[:], pT[:],
                                qkv[:, 2 * E + h * HD:2 * E + (h + 1) * HD],
                                start=True, stop=True)
                            nc.vector.tensor_scalar(attn[:, h * HD:(h + 1) * HD],
                                                    pav[:], rz[:], None, OP.mult)
                        attnT = ta.tile([128, 4 * T], bf16, tag="xhT")
                        transpose_to(attn[:], attnT[:], T, 4, ident_bf)
                        po = tps.tile([128, 512], f32, tag="mm512")
                        for k in range(4):
                            nc.tensor.matmul(po[:], attnT[:, k * T:(k + 1) * T],
                                             wo_w[:, k * E:(k + 1) * E],
                                             start=(k == 0), stop=(k == 3))
                        nc.vector.tensor_tensor(x[:], x[:], po[:], OP.add)
                        yh = layer_norm(x[:], T, E)
                        yhT = ta.tile([128, 4 * T], bf16, tag="xhT")
                        transpose_to(yh[:], yhT[:], T, 4)
                        h1 = ta.tile([T, HID], bf16, tag="h1")
                        for n in range(2):
                            ph = tps.tile([128, 512], f32, tag="mm512")
                            for k in range(4):
                                nc.tensor.matmul(
                                    ph[:], yhT[:, k * T:(k + 1) * T],
                                    w1_w[:, k * HID + n * 512:
                                         k * HID + (n + 1) * 512],
                                    start=(k == 0), stop=(k == 3))
                            h2_ = ta.tile([T, 512], f32, tag="g_h2")
                            nc.scalar.activation(h2_[:], ph[:], AF.Square)
                            c_ = ta.tile([T, 512], f32, tag="g_c")
                            nc.vector.tensor_scalar(c_[:], h2_[:], 0.044715, 1.0,
                                                    OP.mult, OP.add)
                            dd = ta.tile([T, 512], f32, tag="g_d")
                            nc.vector.tensor_tensor(dd[:], c_[:], ph[:], OP.mult)
                            th = ta.tile([T, 512], f32, tag="g_t")
                            nc.scalar.activation(th[:], dd[:], AF.Tanh,
                                                 scale=0.7978845608028654)
                            ff = ta.tile([T, 512], f32, tag="g_f")
                            nc.vector.tensor_scalar(ff[:], th[:], 1.0, 0.5,
                                                    OP.add, OP.mult)
                            nc.vector.tensor_tensor(h1[:, n * 512:(n + 1) * 512],
                                                    ff[:], ph[:], OP.mult)
                        h1T = ta.tile([128, 8 * T], bf16, tag="h1T")
                        transpose_to(h1[:], h1T[:], T, 8, ident_bf)
                        ph2 = tps.tile([128, 512], f32, tag="mm512")
                        for k in range(8):
                            nc.tensor.matmul(ph2[:], h1T[:, k * T:(k + 1) * T],
                                             w2_w[:, k * E:(k + 1) * E],
                                             start=(k == 0), stop=(k == 7))
                        nc.vector.tensor_tensor(x[:], x[:], ph2[:], OP.add)

                # hsT (interleaved 4t+b) + pre_iface h-part
                wifh_sb = twp.tile([128, 4 * 1025], bf16, tag="wqkv")  # reuse slot
                for k in range(4):
                    nc.sync.dma_start(wifh_sb[:, k * 1025:(k + 1) * 1025],
                                      d_wifh[k * 128:(k + 1) * 128, :])
                for b in range(B):
                    for k in range(4):
                        pt = tps.tile([128, 128], f32, tag="ptr")
                        nc.tensor.transpose(pt[:, 0:T],
                                            xs[b][:, k * 128:(k + 1) * 128],
                                            ident[0:T, 0:T])
                        nc.vector.tensor_copy(hsT_chunk(k, b), pt[:, 0:T])
                    tmp = ta.tile([T, 2 * M], bf16, tag="preif_tmp")
                    for n in range(2):
                        ppi = tps.tile([128, 512], f32, tag="mm512")
                        for k in range(4):
                            nc.tensor.matmul(
                                ppi[:], hsT_chunk(k, b),
                                wifh_sb[:, k * 1025 + n * 512:
                                        k * 1025 + (n + 1) * 512],
                                start=(k == 0), stop=False)
                        nc.tensor.matmul(ppi[:], ones_col[:],
                                         bif_sb[:, n * 512:(n + 1) * 512],
                                         start=False, stop=True)
                        nc.vector.tensor_copy(tmp[:, n * 512:(n + 1) * 512],
                                              ppi[:])
                    nc.sync.dma_start(d_preif[b], tmp[:])
                    pag = tps.tile([T, 1], f32, tag="pag")
                    for k in range(4):
                        nc.tensor.matmul(
                            pag[:], hsT_chunk(k, b),
                            wifh_sb[:, k * 1025 + 1024:k * 1025 + 1025],
                            start=(k == 0), stop=False)
                    nc.tensor.matmul(pag[:], ones_col[:], bif_sb[:, 1024:1025],
                                     start=False, stop=True)
                    tmpa = ta.tile([T, 1], f32, tag="preag_tmp")
                    nc.vector.tensor_copy(tmpa[:], pag[:])
                    nc.sync.dma_start(d_preag[b], tmpa[:, 0:1])

            nc.sync.dma_start(pre_ag[:], d_preag[:])

            # ============ Phase S: the collapsed scan ============
            with (
                tc.tile_pool(name="scana", bufs=2) as sa,
                tc.tile_pool(name="spsA", bufs=1, space=PSUM) as spsA,
                tc.tile_pool(name="spsB", bufs=2, space=PSUM) as spsB,
            ):
                def rv_to_hist(t):
                    prt = spsB.tile([128, 16], f32, tag="prvt")
                    for k in range(4):
                        nc.tensor.transpose(prt[:, 4 * k:4 * k + 4],
                                            rv[:, k * 128:(k + 1) * 128],
                                            ident[0:4, 0:4])
                    for k in range(4):
                        nc.vector.tensor_copy(
                            rvh[:, k * 520 + 4 * t:k * 520 + 4 * t + 4],
                            prt[:, 4 * k:4 * k + 4])

                def synth_ln(src):
                    s1 = sa.tile([B, 1], f32, tag="sln_s1")
                    nc.vector.tensor_reduce(s1[:], src[:], AX.X, op=OP.add)
                    mu = sa.tile([B, 1], f32, tag="sln_mu")
                    nc.vector.tensor_scalar(mu[:], s1[:], 1.0 / M, None, OP.mult)
                    xc = sa.tile([B, M], f32, tag="sln_xc")
                    nc.vector.tensor_scalar(xc[:], src[:], mu[:], None,
                                            OP.subtract)
                    s2 = sa.tile([B, 1], f32, tag="sln_s2")
                    sq = sa.tile([B, M], f32, tag="sln_sq")
                    nc.vector.scalar_tensor_tensor(sq[:], xc[:], 1.0, xc[:],
                                                   OP.mult, OP.mult,
                                                   accum_out=s2[:])
                    var = sa.tile([B, 1], f32, tag="sln_var")
                    nc.vector.tensor_scalar(var[:], s2[:], 1.0 / M, None, OP.mult)
                    sd = sa.tile([B, 1], f32, tag="sln_sd")
                    nc.scalar.activation(sd[:], var[:], AF.Sqrt, bias=1e-5)
                    ri = sa.tile([B, 1], f32, tag="sln_ri")
                    nc.vector.reciprocal(ri[:], sd[:])
                    xh = sa.tile([B, M], f32, tag="sln_xh")
                    nc.vector.tensor_scalar(xh[:], xc[:], ri[:], None, OP.mult)
                    return xh

                def small_T(src_ap, Fc, tag):
                    prt = spsB.tile([128, 16], f32, tag="prvt")
                    out = sa.tile([128, 4 * Fc], bf16, tag=tag)
                    for k in range(Fc):
                        nc.tensor.transpose(prt[:, 4 * k:4 * k + 4],
                                            src_ap[:, k * 128:(k + 1) * 128],
                                            ident[0:4, 0:4])
                    nc.vector.tensor_copy(out[:], prt[:, 0:4 * Fc])
                    return out

                for t in range(T):
                    rv_to_hist(t)
                    pifq = sa.tile([B, 2 * M], bf16, tag="pifq")
                    nc.sync.dma_start(pifq[:], d_preif[:, t, :])
                    psif = spsA.tile([B, 2 * M], f32, tag="psif")
                    for n in range(2):
                        for k in range(4):
                            nc.tensor.matmul(
                                psif[:, n * 512:(n + 1) * 512],
                                rvh[:, k * 520 + 4 * t:k * 520 + 4 * t + 4],
                                w_ifr[:, k * 1025 + n * 512:
                                      k * 1025 + (n + 1) * 512],
                                start=(k == 0), stop=False)
                        nc.tensor.matmul(
                            psif[:, n * 512:(n + 1) * 512],
                            ident_bf[0:4, 0:4],
                            pifq[:, n * 512:(n + 1) * 512],
                            start=False, stop=True)
                    psag = spsA.tile([B, 1], f32, tag="psag")
                    for k in range(4):
                        nc.tensor.matmul(
                            psag[:],
                            rvh[:, k * 520 + 4 * t:k * 520 + 4 * t + 4],
                            w_ifr[:, k * 1025 + 1024:k * 1025 + 1025],
                            start=(k == 0), stop=(k == 3))
                    aga = sa.tile([B, 1], f32, tag="aga")
                    nc.vector.tensor_tensor(aga[:], psag[:],
                                            pre_ag[:, t:t + 1], OP.add)
                    ers = sa.tile([B, M], f32, tag="ers")
                    nc.scalar.activation(ers[:], psif[:, M:2 * M], AF.Sigmoid)
                    gts = sa.tile([B, 1], f32, tag="gts")
                    nc.scalar.activation(gts[:], aga[:], AF.Sigmoid)
                    nc.vector.tensor_tensor(rv[:], rv[:], m_t[:], OP.add)
                    u1 = sa.tile([B, M], f32, tag="u1")
                    nc.vector.scalar_tensor_tensor(u1[:], m_t[:], -DECAY / S,
                                                   ers[:], OP.mult, OP.mult)
                    u2 = sa.tile([B, M], f32, tag="u2")
                    nc.vector.scalar_tensor_tensor(u2[:], m_t[:], DECAY, u1[:],
                                                   OP.mult, OP.add)
                    u3 = sa.tile([B, M], f32, tag="u3")
                    nc.vector.tensor_scalar(u3[:], psif[:, 0:M], gts[:],
                                            DECAY / S, OP.mult, OP.mult)
                    nc.vector.tensor_tensor(m_t[:], u2[:], u3[:], OP.add)

                    if t % INTERVAL == 0:
                        xh = synth_ln(m_t)
                        xhT = small_T(xh[:], 4, "xhT_s")
                        py = spsA.tile([B, M], f32, tag="psy")
                        for k in range(4):
                            nc.tensor.matmul(py[:], xhT[:, 4 * k:4 * k + 4],
                                             sv_sb[:, k * M:(k + 1) * M],
                                             start=(k == 0), stop=(k == 3))
                        y1 = sa.tile([B, M], f32, tag="y1")
                        nc.vector.tensor_copy(y1[:], py[:])
                        y1T = small_T(y1[:], 4, "y1T_s")
                        py2 = spsA.tile([B, M], f32, tag="psy")
                        for k in range(4):
                            nc.tensor.matmul(py2[:], y1T[:, 4 * k:4 * k + 4],
                                             so_sb[:, k * M:(k + 1) * M],
                                             start=(k == 0), stop=(k == 3))
                        nc.vector.tensor_tensor(m_t[:], m_t[:], py2[:], OP.add)
                        x2 = synth_ln(m_t)
                        x2T = small_T(x2[:], 4, "x2T_s")
                        ph1 = spsA.tile([B, 128], f32, tag="psh")
                        for k in range(4):
                            nc.tensor.matmul(ph1[:], x2T[:, 4 * k:4 * k + 4],
                                             sw1_sb[:, k * 128:(k + 1) * 128],
                                             start=(k == 0), stop=(k == 3))
                        g2 = sa.tile([B, 128], f32, tag="g2")
                        nc.scalar.activation(g2[:], ph1[:], AF.Square)
                        gc = sa.tile([B, 128], f32, tag="gc")
                        nc.vector.tensor_scalar(gc[:], g2[:], 0.044715, 1.0,
                                                OP.mult, OP.add)
                        gd = sa.tile([B, 128], f32, tag="gd")
                        nc.vector.tensor_tensor(gd[:], gc[:], ph1[:], OP.mult)
                        gt_ = sa.tile([B, 128], f32, tag="gt_")
                        nc.scalar.activation(gt_[:], gd[:], AF.Tanh,
                                             scale=0.7978845608028654)
                        gf = sa.tile([B, 128], f32, tag="gf")
                        nc.vector.tensor_scalar(gf[:], gt_[:], 1.0, 0.5,
                                                OP.add, OP.mult)
                        gg = sa.tile([B, 128], f32, tag="gg")
                        nc.vector.tensor_tensor(gg[:], gf[:], ph1[:], OP.mult)
                        ggT = small_T(gg[:], 1, "ggT_s")
                        pm2 = spsA.tile([B, M], f32, tag="psy")
                        nc.tensor.matmul(pm2[:], ggT[:, 0:4], sw2_sb[:],
                                         start=True, stop=True)
                        nc.vector.tensor_tensor(m_t[:], m_t[:], pm2[:], OP.add)

                rv_to_hist(T)

            # ============ Phase L: logits + recon ============
            with (
                tc.tile_pool(name="lg", bufs=3) as lg,
                tc.tile_pool(name="lgo", bufs=2) as lgo,
                tc.tile_pool(name="lpsum", bufs=1, space=PSUM) as lps,
            ):
                def ctrl_chunk(k, mt):
                    if k < 4:
                        return hsT[:, k * TB + 128 * mt:k * TB + 128 * (mt + 1)]
                    return rvh[:, (k - 4) * 520 + 128 * mt:
                               (k - 4) * 520 + 128 * (mt + 1)]

                nsizes = [512] * 7 + [VS - 7 * 512]
                ncol = 0
                for n, nsz in enumerate(nsizes):
                    plgs = [lps.tile([128, 512], f32, tag=f"plg{mt}", name=f"plg{mt}")
                            for mt in range(4)]
                    for k in range(8):
                        wl = lg.tile([128, 512], bf16, tag="wl")
                        nc.sync.dma_start(wl[:, 0:nsz],
                                          d_wlog[k * 128:(k + 1) * 128,
                                                 ncol:ncol + nsz])
                        for mt in range(4):
                            nc.tensor.matmul(plgs[mt][:, 0:nsz],
                                             ctrl_chunk(k, mt), wl[:, 0:nsz],
                                             start=(k == 0), stop=(k == 7))
                    for mt in range(4):
                        ob = lgo.tile([128, 512], f32, tag="ob")
                        nc.vector.tensor_copy(ob[:, 0:nsz], plgs[mt][:, 0:nsz])
                        nc.sync.dma_start(
                            d_logits[128 * mt:128 * (mt + 1), ncol:ncol + nsz],
                            ob[:, 0:nsz])
                    ncol += nsz

                for mt in range(4):
                    prc = lps.tile([128, E], f32, tag="prc")
                    for k in range(4):
                        nc.tensor.matmul(
                            prc[:],
                            rvh[:, k * 520 + 4 + 128 * mt:
                                k * 520 + 4 + 128 * (mt + 1)],
                            whall_sb[:, k * E:(k + 1) * E],
                            start=(k == 0), stop=False)
                    nc.tensor.matmul(prc[:], ones_col[:], bhall_sb[:],
                                     start=False, stop=True)
                    orc = lgo.tile([128, E], f32, tag="orc")
                    nc.vector.tensor_copy(orc[:], prc[:])
                    nc.sync.dma_start(d_recon[128 * mt:128 * (mt + 1), :],
                                      orc[:])

    nc.compile()
    return nc


def kernel(**inputs):
    inputs = {k: np.asarray(v) for k, v in inputs.items()}
    x0 = (inputs["embed"][inputs["input_seq"].astype(np.int64)]
          + inputs["pos"][None, :T, :]).astype(np.float32)
    maskb = np.where(np.tril(np.ones((T, T), bool)), 0.0,
                     -10000.0).astype(np.float32)
    W_iface = inputs["W_iface"].astype(np.float32)
    base = {
        "x0": x0,
        "Wqkv": inputs["Wqkv"].astype(BF16),
        "Wo": inputs["Wo"].astype(BF16),
        "W1": inputs["W1"].astype(BF16),
        "W2": inputs["W2"].astype(BF16),
        "maskb": maskb,
        "W_if_h": W_iface[:E, 2 * M:].astype(BF16),
        "W_if_r": W_iface[E:, 2 * M:].astype(BF16),
        "b_if": inputs["b_iface"][2 * M:].reshape(1, -1).astype(BF16),
        "Sv": inputs["Sv"][0].astype(BF16),
        "So": inputs["So"][0].astype(BF16),
        "SW1": inputs["SW1"][0].astype(BF16),
        "SW2": inputs["SW2"][0].astype(BF16),
        "W_hall": inputs["W_hall"].astype(BF16),
        "b_hall": inputs["b_hall"].reshape(1, -1).astype(BF16),
    }
    W_logits = inputs["W_logits"].astype(np.float32)
    in_maps = []
    for c in range(N_CORES):
        im = dict(base)
        im["W_log"] = np.ascontiguousarray(W_logits[:, c * VS:(c + 1) * VS]).astype(BF16)
        in_maps.append(im)

    if "nc" not in _COMPILED:
        _COMPILED["nc"] = _build_program()
    from concourse.bass_utils import run_bass_kernel_spmd
    res = run_bass_kernel_spmd(_COMPILED["nc"], in_maps,
                               core_ids=list(range(N_CORES)))
    globals()["LAST_RES"] = res
    logits = np.empty((B, T, V), np.float32)
    for c in range(N_CORES):
        sl = res.results[c]["logits_s"]                  # [TB, VS], row 4t+b
        logits[:, :, c * VS:(c + 1) * VS] = \
            sl.reshape(T, B, VS).transpose(1, 0, 2)
    recon = res.results[0]["recon_s"].reshape(T, B, E).transpose(1, 0, 2)
    return logits, np.ascontiguousarray(recon)
